# revision 9
# baseline (speedup 1.0000x reference)
"""Multi-head attention (B=2, S=2048, H=1024, 16 heads) on 8 TRN2 NeuronCores.

Sharding (tensor-parallel heads x data-parallel batch, per the hint):
  core c -> batch b = c // 4, head group g = c % 4 (4 heads each).

Per-core pipeline (all-fp16 data path):
  phase 1: V^T, Q^T, K^T projections (fp16 matmuls, PSUM f32); Q/K biases
    added on the scalar engine during the PSUM->SBUF copy; V^T transposed
    to natural V (fp16) on the PE with a ones column at row 64 so the ctx
    matmul also produces the softmax denominator.
  phase 2: per (head, chunk-pair) software pipeline:
    scores^T = K^T.T @ Q^T (two tok_k chunks row-tiled concurrently),
    probs = exp(scores) split across TWO engines by (head, tok_q block):
      - ACT tiles: scalar engine exp -> fp16
      - DVE tiles: dual-phase Schraudolph bit trick: two tensor_scalar
        (f32 mult+add -> int16 floor) ops half an octave apart, bitcast
        to fp16, summed on gpsimd.  ~0.5% RMS multiplicative noise;
        each softmax column uses exactly one engine so per-engine scale
        bias cancels in the division.
    ctx^T accumulated over chunks (M=65 ones-augmented V), division by
    the denominator (vector reciprocal + gpsimd partition broadcast +
    gpsimd multiply), heads ordered (1,0,3,2) so the ci=0 half of ctxf
    finishes early; the ci=0 out-projection is interleaved into the
    remaining pipeline stages (PSUM tiles shaped like scores tiles).
  phase 3: ci=1 out-projection.  The two halves go to separate DRAM
    outputs (out0/out1) summed on the host along with the 4 TP partials
    per batch (Megatron-style) and the exact bv@Wo+bo bias term.
"""

import ml_dtypes
import numpy as np

import concourse.bacc as bacc
import concourse.mybir as mybir
import concourse.tile as tile
from concourse.bass_utils import run_bass_kernel_spmd

NCORES = 8
B, S, HID = 2, 2048, 1024
NH, HD = 16, 64
HPC = 4            # heads per core
QC = HPC * HD      # 256 local projection cols per core
HC = HID // 128    # 8 hidden chunks
TC = S // 128      # 16 token chunks
TB = S // 512      # 4 token blocks

F32 = mybir.dt.float32
BF16 = mybir.dt.bfloat16
FP16 = mybir.dt.float16
I16 = mybir.dt.int16
EXP = mybir.ActivationFunctionType.Exp
IDENT_FN = mybir.ActivationFunctionType.Identity
MULT = mybir.AluOpType.mult
ADD = mybir.AluOpType.add

# Dual-phase Schraudolph constants (fp16 bit domain): bits = floor(A*s+B),
# two phases half an octave (512 int16 units) apart, summed.  Scores span
# [-7.3, 6.5] for this problem -> bits in [4000, 25500], far from both the
# subnormal floor and the fp16 Inf boundary.
A16 = 1024.0 / float(np.log(2.0))
B16A = 15360.0
B16B = B16A - 512.0
# engine per (head, tok_q block): True -> DVE Schraudolph, False -> ACT exp.
DVE_EXP = {(h, j): (h * TB + j) in (1, 3, 5, 7, 9, 11, 14)
           for h in range(HPC) for j in range(TB)}


def build_nc():
    nc = bacc.Bacc("TRN2", target_bir_lowering=False, debug=False,
                   num_devices=NCORES)
    xT = nc.declare_dram_parameter("xT", [HID, S], FP16, isOutput=False)
    wq = nc.declare_dram_parameter("wq", [HID, QC], FP16, isOutput=False)
    wk = nc.declare_dram_parameter("wk", [HID, QC], FP16, isOutput=False)
    wv = nc.declare_dram_parameter("wv", [HID, QC], FP16, isOutput=False)
    wo = nc.declare_dram_parameter("wo", [QC, HID], FP16, isOutput=False)
    bq = nc.declare_dram_parameter("bq", [QC], F32, isOutput=False)
    bk = nc.declare_dram_parameter("bk", [QC], F32, isOutput=False)
    ident = nc.declare_dram_parameter("ident", [128, 128], F32, isOutput=False)
    out0 = nc.declare_dram_parameter("out0", [S, HID], FP16, isOutput=True)
    out1 = nc.declare_dram_parameter("out1", [S, HID], FP16, isOutput=True)

    with tile.TileContext(nc) as tc:
        with (
            tc.tile_pool(name="const", bufs=1) as constp,
            tc.tile_pool(name="qkv", bufs=1) as qkvp,
        ):
            wo_sb = constp.tile([128, 2 * HID], FP16)
            bq_sb = constp.tile([128, 2], F32)
            bk_sb = constp.tile([128, 2], F32)
            warm_sb = constp.tile([128, 1], F32)
            idf_sb = constp.tile([128, 128], F32)
            idh_sb = constp.tile([128, 128], FP16)
            # Preload the exp table set while DMAs run (first real exp is
            # on the critical path otherwise; table load costs ~2.7us).
            nc.vector.memset(warm_sb[:, :], 0.0)
            nc.scalar.activation(warm_sb[:, :], warm_sb[:, :], EXP)
            qt2 = qkvp.tile([128, HPC * S], FP16)
            kt2 = qkvp.tile([128, HPC * S], FP16)
            # Natural V (fp16) with a ones column at col 64 of each 128-wide
            # per-head strip: the ctx matmul's M=65 stationary computes ctx
            # rows 0..63 plus the softmax denominator in row 64.
            v_sb = qkvp.tile([128, TC * HPC * 128], FP16)
            vt_sb = qkvp.tile([128, 2 * S], FP16)
            ctxf_sb = qkvp.tile([128, 2 * S], FP16)

            for t in range(TC):
                for h in range(HPC):
                    off = (t * HPC + h) * 128 + HD
                    nc.vector.memset(v_sb[:, off:off + 1], 1.0)

            # ---- phase 1: projections -------------------------------------
            with (
                tc.tile_pool(name="xw", bufs=1) as xwp,
                tc.tile_pool(name="ps1", bufs=2, space="PSUM") as ps1,
            ):
                xT_sb = xwp.tile([128, HC * S], FP16)
                wq_sb = xwp.tile([128, HC * QC], FP16)
                wk_sb = xwp.tile([128, HC * QC], FP16)
                wv_sb = xwp.tile([128, HC * QC], FP16)
                # wv + the first xT chunks get the DMA engines to
                # themselves; later inputs are paced behind early V^T
                # matmuls (add_dep_helper) so the first compute isn't stuck
                # behind the whole 6 MB input load.
                xt_dmas = {}
                for hc in range(HC):
                    r = slice(hc * 128, (hc + 1) * 128)
                    nc.scalar.dma_start(wv_sb[:, hc * QC:(hc + 1) * QC],
                                        wv[r, :])
                    eng = nc.sync if hc % 2 == 0 else nc.scalar
                    if hc == 0:
                        # j-quartered so the first V^T matmul (which reads
                        # only tokens 0..511 of chunk 0) starts asap
                        for j in range(TB):
                            xt_dmas[hc] = eng.dma_start(
                                xT_sb[:, hc * S + j * 512:hc * S + (j + 1) * 512],
                                xT[r, j * 512:(j + 1) * 512])
                    else:
                        xt_dmas[hc] = eng.dma_start(
                            xT_sb[:, hc * S:(hc + 1) * S], xT[r, :])
                nc.scalar.dma_start(idf_sb[:, :], ident[:, :])
                nc.vector.tensor_copy(idh_sb[:, :], idf_sb[:, :])
                for ci in range(2):
                    nc.sync.dma_start(bq_sb[:, ci:ci + 1],
                                      bq[ci * 128:(ci + 1) * 128])
                    nc.sync.dma_start(bk_sb[:, ci:ci + 1],
                                      bk[ci * 128:(ci + 1) * 128])
                qk_dmas = []
                for hc in range(HC):
                    r = slice(hc * 128, (hc + 1) * 128)
                    qk_dmas.append(nc.sync.dma_start(
                        wq_sb[:, hc * QC:(hc + 1) * QC], wq[r, :]))
                    qk_dmas.append(nc.sync.dma_start(
                        wk_sb[:, hc * QC:(hc + 1) * QC], wk[r, :]))

                # V^T first (kept in SBUF; transposed on the PE below)
                vt_mms = {}
                for ci in range(2):
                    ps = ps1.tile([128, S], F32, tag="ps1")
                    for hc in range(HC):
                        for j in range(TB):
                            mm = nc.tensor.matmul(
                                ps[:, j * 512:(j + 1) * 512],
                                wv_sb[:, hc * QC + ci * 128:
                                      hc * QC + ci * 128 + 128],
                                xT_sb[:, hc * S + j * 512:
                                      hc * S + j * 512 + 512],
                                start=(hc == 0), stop=(hc == HC - 1))
                            vt_mms[(ci, hc, j)] = mm
                    nc.vector.tensor_copy(vt_sb[:, ci * S:(ci + 1) * S], ps[:])
                for hc in range(2, HC):
                    tile.add_dep_helper(xt_dmas[hc].ins, vt_mms[(0, hc - 2, 3)].ins,
                                        reason="pace xT input load")
                for i, d in enumerate(qk_dmas):
                    src_mm = vt_mms[(0, min(i // 2, HC - 1), 1)]
                    tile.add_dep_helper(d.ins, src_mm.ins, reason="pace w input load")
                for ci in range(2):
                    d = nc.scalar.dma_start(
                        wo_sb[:, ci * HID:(ci + 1) * HID],
                        wo[ci * 128:(ci + 1) * 128, :])
                    tile.add_dep_helper(d.ins, vt_mms[(1, 3 + 2 * ci, 0)].ins,
                                        reason="pace wo load")

                # Q^T and K^T, written into the duplicated per-head layout.
                # The bias add + PSUM->SBUF copy runs on the scalar engine
                # (Identity with per-partition bias) -- DVE is busy with
                # the V^T copies and V transposes in this phase.
                for ci in range(2):
                    for w_sb, b_sb, dst in ((wq_sb, bq_sb, qt2),
                                            (wk_sb, bk_sb, kt2)):
                        ps = ps1.tile([128, S], F32, tag="ps1")
                        for hc in range(HC):
                            for j in range(TB):
                                nc.tensor.matmul(
                                    ps[:, j * 512:(j + 1) * 512],
                                    w_sb[:, hc * QC + ci * 128:
                                         hc * QC + ci * 128 + 128],
                                    xT_sb[:, hc * S + j * 512:
                                          hc * S + j * 512 + 512],
                                    start=(hc == 0), stop=(hc == HC - 1))
                        hA, hB = 2 * ci, 2 * ci + 1
                        nc.scalar.activation(
                            dst[0:64, hA * S:(hA + 1) * S], ps[0:64, :],
                            IDENT_FN, bias=b_sb[0:64, ci:ci + 1])
                        nc.vector.tensor_scalar_add(
                            dst[64:128, hB * S:(hB + 1) * S], ps[64:128, :],
                            b_sb[64:128, ci:ci + 1])
                        nc.sync.dma_start(dst[64:128, hA * S:(hA + 1) * S],
                                          dst[0:64, hA * S:(hA + 1) * S])
                        nc.scalar.dma_start(dst[0:64, hB * S:(hB + 1) * S],
                                            dst[64:128, hB * S:(hB + 1) * S])

            # V^T -> V via PE transpose-mode ([128,128] pair tiles), then a
            # strided DVE copy into the ones-padded layout.
            with tc.tile_pool(name="trp", bufs=4, space="PSUM") as trp:
                for ci in range(2):
                    for t in range(TC):
                        tp = trp.tile([128, 128], FP16, tag="tr")
                        nc.tensor.transpose(
                            tp[:, :],
                            vt_sb[:, ci * S + t * 128:ci * S + t * 128 + 128],
                            idh_sb[:, :])
                        dst = v_sb[:, (t * HPC + 2 * ci) * 128:
                                   (t * HPC + 2 * ci + 2) * 128].rearrange(
                            "p (h e) -> p h e", h=2)[:, :, 0:HD]
                        srcv = tp[:, :].rearrange("p (h e) -> p h e", h=2)
                        nc.vector.tensor_copy(dst, srcv)

            # ---- phase 2: attention per head + interleaved ci0 out-proj ---
            with (
                tc.tile_pool(name="probs", bufs=5) as probsp,
                tc.tile_pool(name="schr", bufs=4) as schrp,
                tc.tile_pool(name="craw", bufs=2) as crawp,
                tc.tile_pool(name="div", bufs=2) as divp,
                tc.tile_pool(name="ostg", bufs=3) as ostg,
                tc.tile_pool(name="scps", bufs=2, space="PSUM") as scps,
                tc.tile_pool(name="ctps", bufs=1, space="PSUM") as ctps,
            ):
                NCP = TC // 2
                heads = (1, 0, 3, 2)
                probs_tiles = {}
                ctx_tiles = {}

                def emit_scores(h, cp):
                    hS = h * S
                    c0, c1 = 2 * cp, 2 * cp + 1
                    probs_c = probsp.tile([128, 2 * S], FP16, tag="probs",
                                          name=f"probs_h{h}_cp{cp}")
                    probs_tiles[(h, cp)] = probs_c
                    for j in range(TB):
                        sp = scps.tile([128, 1024], F32, tag="sc")
                        nc.tensor.matmul(
                            sp[:, 0:512],
                            kt2[0:64, hS + c0 * 128:hS + c0 * 128 + 128],
                            qt2[0:64, hS + j * 512:hS + j * 512 + 512],
                            start=True, stop=True)
                        nc.tensor.matmul(
                            sp[:, 512:1024],
                            kt2[64:128, hS + c1 * 128:hS + c1 * 128 + 128],
                            qt2[64:128, hS + j * 512:hS + j * 512 + 512],
                            start=True, stop=True)
                        dst = probs_c[:, j * 1024:(j + 1) * 1024]
                        if DVE_EXP[(h, j)]:
                            s1 = schrp.tile([128, 1024], I16, tag="schr")
                            nc.vector.tensor_scalar(
                                s1[:, :], sp[:, :], A16, B16A, MULT, ADD)
                            nc.vector.tensor_scalar(
                                dst.bitcast(I16), sp[:, :], A16, B16B,
                                MULT, ADD)
                            nc.gpsimd.tensor_tensor(
                                out=dst, in0=dst, in1=s1[:, :].bitcast(FP16),
                                op=ADD)
                        else:
                            nc.scalar.activation(dst, sp[:, :], EXP)

                def emit_ctx(h, cp):
                    c0, c1 = 2 * cp, 2 * cp + 1
                    if cp == 0:
                        ctx_tiles[h] = ctps.tile([128, S], F32, tag="ctx",
                                                 name=f"ctx_ps_h{h}")
                    ctx_ps = ctx_tiles[h]
                    probs_c = probs_tiles.pop((h, cp))
                    for ck, coff in ((c0, 0), (c1, 512)):
                        vbase = (ck * HPC + h) * 128
                        for j in range(TB):
                            nc.tensor.matmul(
                                ctx_ps[0:65, j * 512:(j + 1) * 512],
                                v_sb[:, vbase:vbase + 65],
                                probs_c[:, j * 1024 + coff:
                                        j * 1024 + coff + 512],
                                start=(cp == 0 and ck == c0),
                                stop=(cp == NCP - 1 and ck == c1))

                def emit_division(h):
                    ci = h // 2
                    ctx_ps = ctx_tiles.pop(h)
                    craw = crawp.tile([128, S], F32, tag="craw")
                    nc.vector.tensor_copy(craw[0:65, :], ctx_ps[0:65, :])
                    denr = divp.tile([128, 16], F32, tag="denr")
                    nc.sync.dma_start(denr[:, :], craw[64:65, :])
                    recr = divp.tile([128, 16], F32, tag="recr")
                    nc.vector.reciprocal(recr[:], denr[:])
                    rrow = divp.tile([1, S], F32, tag="rrow")
                    nc.sync.dma_start(rrow[:, :], recr[:, :])
                    Dt = divp.tile([128, S], F32, tag="Dt")
                    nc.gpsimd.partition_broadcast(Dt[:, :], rrow[0:1, :])
                    if h % 2 == 0:
                        nc.gpsimd.tensor_tensor(
                            out=ctxf_sb[0:64, ci * S:(ci + 1) * S],
                            in0=craw[0:64, :], in1=Dt[0:64, :], op=MULT)
                    else:
                        ctxd = crawp.tile([64, S], FP16, tag="ctxd")
                        nc.gpsimd.tensor_tensor(
                            out=ctxd[0:64, :],
                            in0=craw[0:64, :], in1=Dt[0:64, :], op=MULT)
                        nc.sync.dma_start(
                            ctxf_sb[64:128, ci * S:(ci + 1) * S],
                            ctxd[0:64, :])

                def emit_outproj(ci, t, out_t):
                    op = scps.tile([128, 1024], F32, tag="sc",
                                   name=f"op_ci{ci}_t{t}")
                    for oc in range(2):
                        nc.tensor.matmul(
                            op[:, oc * 512:(oc + 1) * 512],
                            ctxf_sb[:, ci * S + t * 128:ci * S + t * 128 + 128],
                            wo_sb[:, ci * HID + oc * 512:
                                  ci * HID + oc * 512 + 512],
                            start=True, stop=True)
                    ot = ostg.tile([128, 1024], FP16, tag="ot")
                    nc.vector.tensor_copy(ot[:, :], op[:, :])
                    nc.sync.dma_start(out_t[t * 128:(t + 1) * 128, :], ot[:, :])

                # software pipeline: scores/exp lead ctx by LEAD cp-steps so
                # the PE stream keeps flowing across head boundaries.  After
                # the ci=0 half (heads 1,0) completes, one ci=0 out-proj
                # token chunk is interleaved after every remaining stage.
                LEAD = 2
                stages = [(h, cp) for h in heads for cp in range(NCP)]
                op0_t = 0
                for i in range(len(stages) + LEAD):
                    if i < len(stages):
                        emit_scores(*stages[i])
                    if i >= LEAD:
                        h, cp = stages[i - LEAD]
                        emit_ctx(h, cp)
                        if cp == NCP - 1:
                            emit_division(h)
                        if i - LEAD >= 2 * NCP and op0_t < TC:
                            emit_outproj(0, op0_t, out0)
                            op0_t += 1
                while op0_t < TC:
                    emit_outproj(0, op0_t, out0)
                    op0_t += 1

            # ---- phase 3: ci=1 out projection ----------------------------
            with (
                tc.tile_pool(name="ops", bufs=4, space="PSUM") as ops,
                tc.tile_pool(name="ostg2", bufs=3) as ostg2,
            ):
                for t in range(TC):
                    op = ops.tile([128, 1024], F32, tag="op")
                    for oc in range(2):
                        nc.tensor.matmul(
                            op[:, oc * 512:(oc + 1) * 512],
                            ctxf_sb[:, S + t * 128:S + t * 128 + 128],
                            wo_sb[:, HID + oc * 512:HID + oc * 512 + 512],
                            start=True, stop=True)
                    ot = ostg2.tile([128, 1024], FP16, tag="ot")
                    nc.vector.tensor_copy(ot[:, :], op[:, :])
                    nc.sync.dma_start(out1[t * 128:(t + 1) * 128, :], ot[:, :])

    nc.compile()
    return nc


_NC = None


def _get_nc():
    global _NC
    if _NC is None:
        _NC = build_nc()
    return _NC


def make_in_maps(x, Wq, bq, Wk, bk, Wv, bv, Wo, bo):
    in_maps = []
    for core in range(NCORES):
        b, g = core // 4, core % 4
        sl = slice(g * QC, (g + 1) * QC)
        in_maps.append({
            "xT": np.ascontiguousarray(x[b].T).astype(np.float16),
            "wq": (np.ascontiguousarray(Wq[:, sl]) * 0.125).astype(np.float16),
            "wk": np.ascontiguousarray(Wk[:, sl]).astype(np.float16),
            "wv": np.ascontiguousarray(Wv[:, sl]).astype(np.float16),
            "wo": np.ascontiguousarray(Wo[sl, :]).astype(np.float16),
            "bq": (np.asarray(bq[sl]) * 0.125).astype(np.float32),
            "bk": np.asarray(bk[sl]).astype(np.float32),
            "ident": np.eye(128, dtype=np.float32),
        })
    return in_maps


def combine_outputs(core_outs, Wv_bias_term):
    full = np.empty((B, S, HID), np.float32)
    for b in range(B):
        acc = core_outs[4 * b][0].astype(np.float32)
        acc += core_outs[4 * b][1]
        for g in range(1, 4):
            acc += core_outs[4 * b + g][0]
            acc += core_outs[4 * b + g][1]
        full[b] = acc + Wv_bias_term
    return full


def kernel(**inputs):
    x = np.asarray(inputs["x"], np.float32)
    Wq = np.asarray(inputs["Wq"], np.float32)
    bq = np.asarray(inputs["bq"], np.float32)
    Wk = np.asarray(inputs["Wk"], np.float32)
    bk = np.asarray(inputs["bk"], np.float32)
    Wv = np.asarray(inputs["Wv"], np.float32)
    bv = np.asarray(inputs["bv"], np.float32)
    Wo = np.asarray(inputs["Wo"], np.float32)
    bo = np.asarray(inputs["bo"], np.float32)

    nc = _get_nc()
    in_maps = make_in_maps(x, Wq, bq, Wk, bk, Wv, bv, Wo, bo)
    res = run_bass_kernel_spmd(nc, in_maps, core_ids=list(range(NCORES)))
    core_outs = [(res.results[c]["out0"], res.results[c]["out1"])
                 for c in range(NCORES)]
    bias_term = (bv @ Wo + bo).astype(np.float32)
    return combine_outputs(core_outs, bias_term)


# revision 18
# speedup vs baseline: 1.0543x; 1.0543x over previous
"""Multi-head attention (B=2, S=2048, H=1024, 16 heads) on 8 TRN2 NeuronCores.

Sharding (tensor-parallel heads x data-parallel batch, per the hint):
  core c -> batch b = c // 4, head group g = c % 4 (4 heads each).

Per-core structure (single software-pipelined loop, all-fp16 data path):
  - Q^T/K^T for the ci=0 heads are projected first (j-blocked [128,512]
    PSUM tiles) so attention scores start as soon as the x^T load lands.
  - 64 pipeline stages (head, tok_q half, chunk pair): scores^T (row-tiled
    fp16 matmul pairs), exp split between the scalar engine (ACT exp,
    scale=1/A16) and the vector engine (dual-phase Schraudolph: two
    single-op adds -> int16 truncate = fp16 bit patterns half an octave
    apart, summed on gpsimd; ~0.5% RMS, column-consistent so the scale
    bias cancels in the softmax division).  The scores matmul computes
    A16*s directly (A16 folded into Wq on the host).
  - ctx^T accumulates in a 2-bank [128,1024] PSUM tile per (head, half);
    the stationary is a two-block AP over natural V plus a shared ones
    region, so row 64 of the output is the softmax denominator (rows
    65..127 are don't-care duplicates).
  - The remaining projections (V^T, Q/K ci=1), PE transposes of V, and
    both out-projection halves are interleaved into the pipeline as
    filler work, so the PE never idles and the HAM clock stays at 8/8.
  - Outputs: two fp16 DRAM tensors (ci=0 / ci=1 out-projection halves,
    Megatron-style partial sums) combined on the host with bv@Wo+bo.
"""

import ml_dtypes
import numpy as np

import concourse.bacc as bacc
import concourse.mybir as mybir
import concourse.tile as tile
from concourse.ap import AP
from concourse.bass_utils import run_bass_kernel_spmd

NCORES = 8
B, S, HID = 2, 2048, 1024
NH, HD = 16, 64
HPC = 4            # heads per core
QC = HPC * HD      # 256 local projection cols per core
HC = HID // 128    # 8 hidden chunks
TC = S // 128      # 16 token chunks
TB = S // 512      # 4 token blocks

F32 = mybir.dt.float32
BF16 = mybir.dt.bfloat16
FP16 = mybir.dt.float16
I16 = mybir.dt.int16
EXP = mybir.ActivationFunctionType.Exp
IDENT_FN = mybir.ActivationFunctionType.Identity
COPY_FN = mybir.ActivationFunctionType.Copy
MULT = mybir.AluOpType.mult
ADD = mybir.AluOpType.add

# Dual-phase Schraudolph constants.  Scores arrive in PSUM pre-scaled by
# A16 = 1024/ln2 (folded into Wq host-side), so bits = trunc(psum + B).
# Scores span [-7.3, 6.5] -> bits in [4500, 25500]: no sign/Inf hazards.
A16 = 1024.0 / float(np.log(2.0))
B16A = 15360.0
B16B = B16A - 512.0
# (head, tok_q block) column groups handled by the DVE instead of ACT.
DVE_GROUPS = {(1, 2), (0, 1), (3, 2), (2, 1), (1, 0), (2, 3), (0, 3)}
DVE_EXP = {(h, j): (h, j) in DVE_GROUPS
           for h in range(HPC) for j in range(TB)}


def build_nc():
    nc = bacc.Bacc("TRN2", target_bir_lowering=False, debug=False,
                   num_devices=NCORES)
    xT = nc.declare_dram_parameter("xT", [HID, S], FP16, isOutput=False)
    wq = nc.declare_dram_parameter("wq", [HID, QC], FP16, isOutput=False)
    wk = nc.declare_dram_parameter("wk", [HID, QC], FP16, isOutput=False)
    wv = nc.declare_dram_parameter("wv", [HID, QC], FP16, isOutput=False)
    wo = nc.declare_dram_parameter("wo", [QC, HID], FP16, isOutput=False)
    bq = nc.declare_dram_parameter("bq", [QC], F32, isOutput=False)
    bk = nc.declare_dram_parameter("bk", [QC], F32, isOutput=False)
    ident = nc.declare_dram_parameter("ident", [128, 128], FP16, isOutput=False)
    out0 = nc.declare_dram_parameter("out0", [S, HID], FP16, isOutput=True)
    out1 = nc.declare_dram_parameter("out1", [S, HID], FP16, isOutput=True)

    with tile.TileContext(nc) as tc:
        with (
            tc.tile_pool(name="const", bufs=1) as constp,
            tc.tile_pool(name="big", bufs=1) as bigp,
            tc.tile_pool(name="vtst", bufs=3) as vtstp,
            tc.tile_pool(name="probs", bufs=6) as probsp,
            tc.tile_pool(name="schr", bufs=4) as schrp,
            tc.tile_pool(name="craw", bufs=3) as crawp,
            tc.tile_pool(name="div", bufs=2) as divp,
            tc.tile_pool(name="ostg", bufs=4) as ostg,
            tc.tile_pool(name="scps", bufs=2, space="PSUM") as scps,
            tc.tile_pool(name="ctps", bufs=1, space="PSUM") as ctps,
            tc.tile_pool(name="wkps", bufs=2, space="PSUM") as wkps,
        ):
            bq_sb = constp.tile([128, 2], F32)
            bk_sb = constp.tile([128, 2], F32)
            warm_sb = constp.tile([128, 1], F32)
            idh_sb = constp.tile([128, 128], FP16)
            wo_sb = constp.tile([128, 2 * HID], FP16)
            # preload the exp table set while the input DMAs run
            nc.vector.memset(warm_sb[:, :], 0.0)
            nc.scalar.activation(warm_sb[:, :], warm_sb[:, :], EXP)

            xT_sb = bigp.tile([128, HC * S], FP16)
            wq_sb = bigp.tile([128, HC * QC], FP16)
            wk_sb = bigp.tile([128, HC * QC], FP16)
            wv_sb = bigp.tile([128, HC * QC], FP16)
            qt2 = bigp.tile([128, HPC * S], FP16)
            kt2 = bigp.tile([128, HPC * S], FP16)
            # natural V in 128-wide per-(chunk, head) strips: cols 0:64 hold
            # the head's V dims, col 64 holds ones (so the M=65 ctx matmul
            # stationary also produces the softmax denominator in row 64).
            v_sb = bigp.tile([128, TC * HPC * 128], FP16)
            ctxf_sb = bigp.tile([128, 2 * S], FP16)
            for t in range(TC):
                for h in range(HPC):
                    off = (t * HPC + h) * 128 + HD
                    nc.vector.memset(v_sb[:, off:off + 1], 1.0)

            # ---- input DMAs: xT spread over 3 queues, ci0 weights first --
            qdma = [nc.sync, nc.scalar, nc.gpsimd, nc.sync]
            for hc in range(HC):
                r = slice(hc * 128, (hc + 1) * 128)
                eng = qdma[hc % 4]
                eng.dma_start(wq_sb[:, hc * QC:hc * QC + 128], wq[r, 0:128])
                eng.dma_start(wk_sb[:, hc * QC:hc * QC + 128], wk[r, 0:128])
                eng.dma_start(xT_sb[:, hc * S:(hc + 1) * S], xT[r, :])
            nc.sync.dma_start(idh_sb[:, :], ident[:, :])
            for ci in range(2):
                nc.sync.dma_start(bq_sb[:, ci:ci + 1],
                                  bq[ci * 128:(ci + 1) * 128])
                nc.sync.dma_start(bk_sb[:, ci:ci + 1],
                                  bk[ci * 128:(ci + 1) * 128])
            # later-needed weights: queue after the critical loads
            for hc in range(HC):
                r = slice(hc * 128, (hc + 1) * 128)
                qdma[hc % 4].dma_start(wv_sb[:, hc * QC:(hc + 1) * QC],
                                       wv[r, :])
            for hc in range(HC):
                r = slice(hc * 128, (hc + 1) * 128)
                qdma[(hc + 1) % 4].dma_start(
                    wq_sb[:, hc * QC + 128:hc * QC + 256], wq[r, 128:256])
                qdma[(hc + 2) % 4].dma_start(
                    wk_sb[:, hc * QC + 128:hc * QC + 256], wk[r, 128:256])
            for ci in range(2):
                qdma[1 + ci].dma_start(wo_sb[:, ci * HID:(ci + 1) * HID],
                                       wo[ci * 128:(ci + 1) * 128, :])

            # ---- building blocks -----------------------------------------
            def qk_unit(w_sb, b_sb, dst, ci, j):
                """Project one j-block of Q^T or K^T for ci's head pair into
                the duplicated per-head layout."""
                ps = wkps.tile([128, 512], F32, tag="wk")
                for hc in range(HC):
                    nc.tensor.matmul(
                        ps[:, :],
                        w_sb[:, hc * QC + ci * 128:hc * QC + ci * 128 + 128],
                        xT_sb[:, hc * S + j * 512:hc * S + j * 512 + 512],
                        start=(hc == 0), stop=(hc == HC - 1))
                hA, hB = 2 * ci, 2 * ci + 1
                jo = j * 512
                nc.scalar.activation(
                    dst[0:64, hA * S + jo:hA * S + jo + 512], ps[0:64, :],
                    IDENT_FN, bias=b_sb[0:64, ci:ci + 1])
                nc.vector.tensor_scalar_add(
                    dst[64:128, hB * S + jo:hB * S + jo + 512], ps[64:128, :],
                    b_sb[64:128, ci:ci + 1])
                nc.sync.dma_start(dst[64:128, hA * S + jo:hA * S + jo + 512],
                                  dst[0:64, hA * S + jo:hA * S + jo + 512])
                nc.scalar.dma_start(dst[0:64, hB * S + jo:hB * S + jo + 512],
                                    dst[64:128, hB * S + jo:hB * S + jo + 512])

            def vt_unit(ci, j):
                """V^T for one j-block of ci's head pair -> fp16 staging."""
                ps = wkps.tile([128, 512], F32, tag="wk")
                for hc in range(HC):
                    nc.tensor.matmul(
                        ps[:, :],
                        wv_sb[:, hc * QC + ci * 128:hc * QC + ci * 128 + 128],
                        xT_sb[:, hc * S + j * 512:hc * S + j * 512 + 512],
                        start=(hc == 0), stop=(hc == HC - 1))
                vt = vtstp.tile([128, 512], FP16, tag="vt",
                                name=f"vt_{ci}_{j}")
                nc.vector.tensor_copy(vt[:, :], ps[:, :])
                return vt

            def tr_unit(ci, t, vt):
                """Transpose one 128-token chunk of V^T into natural V: two
                contiguous 64-wide copies into the per-head strips."""
                tp = wkps.tile([128, 128], FP16, tag="wk")
                nc.tensor.transpose(tp[:, :],
                                    vt[:, (t % 4) * 128:(t % 4) * 128 + 128],
                                    idh_sb[:, :])
                for hl in range(2):
                    dst = (t * HPC + 2 * ci + hl) * 128
                    nc.vector.tensor_copy(v_sb[:, dst:dst + HD],
                                          tp[:, hl * HD:(hl + 1) * HD])

            def filler_units():
                for ci in range(2):
                    if ci == 1:
                        for j in range(TB):
                            yield lambda j=j: qk_unit(wq_sb, bq_sb, qt2, 1, j)
                            yield lambda j=j: qk_unit(wk_sb, bk_sb, kt2, 1, j)
                    for j in range(TB):
                        vt = []
                        yield lambda ci=ci, j=j, vt=vt: vt.append(
                            vt_unit(ci, j))
                        for tt in range(2):
                            yield lambda ci=ci, j=j, tt=tt, vt=vt: (
                                tr_unit(ci, j * 4 + 2 * tt, vt[0]),
                                tr_unit(ci, j * 4 + 2 * tt + 1, vt[0]))

            def v_stationary(ck, h):
                """[128, 65] stationary: head h's V dims for chunk ck plus
                the ones column -> ctx rows 0:64 + denominator in row 64."""
                base = (ck * HPC + h) * 128
                return v_sb[:, base:base + 65]

            probs_tiles = {}
            ctx_tiles = {}

            def emit_scores(h, jp, cp):
                hS = h * S
                c0, c1 = 2 * cp, 2 * cp + 1
                probs_c = probsp.tile([128, 2048], FP16, tag="probs",
                                      name=f"probs_h{h}_jp{jp}_cp{cp}")
                probs_tiles[(h, jp, cp)] = probs_c
                for jl in range(2):
                    j = 2 * jp + jl
                    sp = scps.tile([128, 1024], F32, tag="sc")
                    nc.tensor.matmul(
                        sp[:, 0:512],
                        kt2[0:64, hS + c0 * 128:hS + c0 * 128 + 128],
                        qt2[0:64, hS + j * 512:hS + j * 512 + 512],
                        start=True, stop=True)
                    nc.tensor.matmul(
                        sp[:, 512:1024],
                        kt2[64:128, hS + c1 * 128:hS + c1 * 128 + 128],
                        qt2[64:128, hS + j * 512:hS + j * 512 + 512],
                        start=True, stop=True)
                    dst = probs_c[:, jl * 1024:(jl + 1) * 1024]
                    if DVE_EXP[(h, j)]:
                        s1 = schrp.tile([128, 1024], I16, tag="schr")
                        nc.vector.tensor_scalar(
                            s1[:, :], sp[:, :], B16A, None, ADD)
                        nc.vector.tensor_scalar(
                            dst.bitcast(I16), sp[:, :], B16B, None, ADD)
                        nc.gpsimd.tensor_tensor(
                            out=dst, in0=dst, in1=s1[:, :].bitcast(FP16),
                            op=ADD)
                    else:
                        nc.scalar.activation(dst, sp[:, :], EXP,
                                             scale=1.0 / A16)

            def emit_ctx(h, jp, cp):
                c0, c1 = 2 * cp, 2 * cp + 1
                if cp == 0:
                    ctx_tiles[(h, jp)] = ctps.tile(
                        [128, 1024], F32, tag="ctx", name=f"ctx_h{h}_jp{jp}")
                ctx_ps = ctx_tiles[(h, jp)]
                probs_c = probs_tiles.pop((h, jp, cp))
                for ck, coff in ((c0, 0), (c1, 512)):
                    lhsT = v_stationary(ck, h)
                    for jl in range(2):
                        nc.tensor.matmul(
                            ctx_ps[0:65, jl * 512:(jl + 1) * 512],
                            lhsT,
                            probs_c[:, jl * 1024 + coff:jl * 1024 + coff + 512],
                            start=(cp == 0 and ck == c0),
                            stop=(cp == TC // 2 - 1 and ck == c1))

            def emit_division(h, jp):
                ci = h // 2
                ctx_ps = ctx_tiles.pop((h, jp))
                craw = crawp.tile([128, 1024], F32, tag="craw")
                nc.vector.tensor_copy(craw[0:65, :], ctx_ps[0:65, :])
                denr = divp.tile([128, 8], F32, tag="denr")
                nc.sync.dma_start(denr[:, :], craw[64:65, :])
                recr = divp.tile([128, 8], F32, tag="recr")
                nc.vector.reciprocal(recr[:], denr[:])
                rrow = divp.tile([1, 1024], F32, tag="rrow")
                nc.sync.dma_start(rrow[:, :], recr[:, :])
                Dt = divp.tile([128, 1024], F32, tag="Dt")
                nc.gpsimd.partition_broadcast(Dt[:, :], rrow[0:1, :])
                dst_c = ci * S + jp * 1024
                if h % 2 == 0:
                    nc.gpsimd.tensor_tensor(
                        out=ctxf_sb[0:64, dst_c:dst_c + 1024],
                        in0=craw[0:64, :], in1=Dt[0:64, :], op=MULT)
                else:
                    ctxd = crawp.tile([64, 1024], FP16, tag="ctxd")
                    nc.gpsimd.tensor_tensor(
                        out=ctxd[0:64, :],
                        in0=craw[0:64, :], in1=Dt[0:64, :], op=MULT)
                    nc.sync.dma_start(
                        ctxf_sb[64:128, dst_c:dst_c + 1024], ctxd[0:64, :])

            def emit_outproj(ci, t, out_t, copy_eng):
                ot = ostg.tile([128, 1024], FP16, tag="ot")
                for oc in range(2):
                    op = wkps.tile([128, 512], F32, tag="wk")
                    nc.tensor.matmul(
                        op[:, :],
                        ctxf_sb[:, ci * S + t * 128:ci * S + t * 128 + 128],
                        wo_sb[:, ci * HID + oc * 512:ci * HID + oc * 512 + 512],
                        start=True, stop=True)
                    if copy_eng == "act":
                        nc.scalar.activation(
                            ot[:, oc * 512:(oc + 1) * 512], op[:, :], COPY_FN)
                    else:
                        nc.vector.tensor_copy(
                            ot[:, oc * 512:(oc + 1) * 512], op[:, :])
                nc.sync.dma_start(out_t[t * 128:(t + 1) * 128, :], ot[:, :])

            # ---- emission ------------------------------------------------
            # ci0 Q/K first so scores start as soon as xT lands
            for j in range(TB):
                qk_unit(wq_sb, bq_sb, qt2, 0, j)
                qk_unit(wk_sb, bk_sb, kt2, 0, j)

            # stage list: ci0 heads first; the last head pair interleaved by
            # jp so the ci1/jp0 quarter of ctxf completes early.
            NCP = TC // 2
            stages = ([(1, jp, cp) for jp in range(2) for cp in range(NCP)] +
                      [(0, jp, cp) for jp in range(2) for cp in range(NCP)] +
                      [(3, 0, cp) for cp in range(NCP)] +
                      [(2, 0, cp) for cp in range(NCP)] +
                      [(3, 1, cp) for cp in range(NCP)] +
                      [(2, 1, cp) for cp in range(NCP)])
            fillers = filler_units()
            LEAD = 2
            nslot = 0

            def pump_outproj(i):
                nonlocal nslot
                # ci0 ready after stage 31 (+LEAD+division latency), ci1/jp0
                # after stage 47, ci1/jp1 after stage 63.
                if i >= 36 and nslot < 16:
                    t = nslot
                    emit_outproj(0, t, out0, "act" if t % 2 else "dve")
                    nslot += 1
                elif i >= 52 and nslot < 24:
                    t = nslot - 16
                    emit_outproj(1, t, out1, "act" if t % 2 else "dve")
                    nslot += 1

            filler_budget = [0, 2, 2, 2, 2, 2] + [1] * 44
            for i in range(len(stages) + LEAD):
                if i < len(stages):
                    emit_scores(*stages[i])
                    nb = filler_budget[i] if i < len(filler_budget) else 0
                    for _ in range(nb):
                        u = next(fillers, None)
                        if u is not None:
                            u()
                if i >= LEAD:
                    h, jp, cp = stages[i - LEAD]
                    emit_ctx(h, jp, cp)
                    if cp == NCP - 1:
                        emit_division(h, jp)
                    pump_outproj(i)
                    pump_outproj(i)
            for u in fillers:
                u()
            k = 0
            while nslot < 32:
                t = nslot - 16
                emit_outproj(1, t, out1, "act" if k % 2 else "dve")
                nslot += 1
                k += 1

    nc.compile()
    return nc


_NC = None


def _get_nc():
    global _NC
    if _NC is None:
        _NC = build_nc()
    return _NC


def make_in_maps(x, Wq, bq, Wk, bk, Wv, bv, Wo, bo):
    qscale = 0.125 * A16
    in_maps = []
    for core in range(NCORES):
        b, g = core // 4, core % 4
        sl = slice(g * QC, (g + 1) * QC)
        in_maps.append({
            "xT": np.ascontiguousarray(x[b].T).astype(np.float16),
            "wq": (np.ascontiguousarray(Wq[:, sl]) * qscale).astype(np.float16),
            "wk": np.ascontiguousarray(Wk[:, sl]).astype(np.float16),
            "wv": np.ascontiguousarray(Wv[:, sl]).astype(np.float16),
            "wo": np.ascontiguousarray(Wo[sl, :]).astype(np.float16),
            "bq": (np.asarray(bq[sl]) * qscale).astype(np.float32),
            "bk": np.asarray(bk[sl]).astype(np.float32),
            "ident": np.eye(128, dtype=np.float16),
        })
    return in_maps


def combine_outputs(core_outs, Wv_bias_term):
    full = np.empty((B, S, HID), np.float32)
    for b in range(B):
        acc = core_outs[4 * b][0].astype(np.float32)
        acc += core_outs[4 * b][1]
        for g in range(1, 4):
            acc += core_outs[4 * b + g][0]
            acc += core_outs[4 * b + g][1]
        full[b] = acc + Wv_bias_term
    return full


def kernel(**inputs):
    x = np.asarray(inputs["x"], np.float32)
    Wq = np.asarray(inputs["Wq"], np.float32)
    bq = np.asarray(inputs["bq"], np.float32)
    Wk = np.asarray(inputs["Wk"], np.float32)
    bk = np.asarray(inputs["bk"], np.float32)
    Wv = np.asarray(inputs["Wv"], np.float32)
    bv = np.asarray(inputs["bv"], np.float32)
    Wo = np.asarray(inputs["Wo"], np.float32)
    bo = np.asarray(inputs["bo"], np.float32)

    nc = _get_nc()
    in_maps = make_in_maps(x, Wq, bq, Wk, bk, Wv, bv, Wo, bo)
    res = run_bass_kernel_spmd(nc, in_maps, core_ids=list(range(NCORES)))
    core_outs = [(res.results[c]["out0"], res.results[c]["out1"])
                 for c in range(NCORES)]
    bias_term = (bv @ Wo + bo).astype(np.float32)
    return combine_outputs(core_outs, bias_term)


# revision 24
# speedup vs baseline: 1.4549x; 1.3800x over previous
"""Multi-head attention (B=2, S=2048, H=1024, 16 heads) on 8 TRN2 NeuronCores.

Sharding (tensor-parallel heads x data-parallel batch, per the hint):
  core c -> batch b = c // 4, head group g = c % 4 (4 heads each).

Per-core structure (single software-pipelined loop, all-fp16 data path):
  - Q^T/K^T for the ci=0 heads are projected first (j-blocked [128,512]
    PSUM tiles) so attention scores start as soon as the x^T load lands.
  - 64 pipeline stages (head, tok_q half, chunk pair): scores^T (row-tiled
    fp16 matmul pairs), exp split between the scalar engine (ACT exp,
    scale=1/A16) and the vector engine (dual-phase Schraudolph: two
    single-op adds -> int16 truncate = fp16 bit patterns half an octave
    apart, summed on gpsimd; ~0.5% RMS, column-consistent so the scale
    bias cancels in the softmax division).  The scores matmul computes
    A16*s directly (A16 folded into Wq on the host).
  - ctx^T accumulates in a 2-bank [128,1024] PSUM tile per (head, half);
    the stationary is a two-block AP over natural V plus a shared ones
    region, so row 64 of the output is the softmax denominator (rows
    65..127 are don't-care duplicates).
  - The remaining projections (V^T, Q/K ci=1), PE transposes of V, and
    both out-projection halves are interleaved into the pipeline as
    filler work, so the PE never idles and the HAM clock stays at 8/8.
  - Outputs: two fp16 DRAM tensors (ci=0 / ci=1 out-projection halves,
    Megatron-style partial sums) combined on the host with bv@Wo+bo.
"""

import ml_dtypes
import numpy as np

import concourse.bacc as bacc
import concourse.mybir as mybir
import concourse.tile as tile
from concourse.ap import AP
from concourse.bass_utils import run_bass_kernel_spmd

NCORES = 8
B, S, HID = 2, 2048, 1024
NH, HD = 16, 64
HPC = 4            # heads per core
QC = HPC * HD      # 256 local projection cols per core
HC = HID // 128    # 8 hidden chunks
TC = S // 128      # 16 token chunks
TB = S // 512      # 4 token blocks

F32 = mybir.dt.float32
BF16 = mybir.dt.bfloat16
FP16 = mybir.dt.float16
I16 = mybir.dt.int16
EXP = mybir.ActivationFunctionType.Exp
IDENT_FN = mybir.ActivationFunctionType.Identity
COPY_FN = mybir.ActivationFunctionType.Copy
MULT = mybir.AluOpType.mult
ADD = mybir.AluOpType.add

# Dual-phase Schraudolph constants.  Scores arrive in PSUM pre-scaled by
# A16 = 1024/ln2 (folded into Wq host-side), so bits = trunc(psum + B).
# Scores span [-7.3, 6.5] -> bits in [4500, 25500]: no sign/Inf hazards.
A16 = 1024.0 / float(np.log(2.0))
B16A = 15360.0
B16B = B16A - 512.0
# (head, tok_q half) column groups handled by the DVE instead of ACT.
# Measured rates: ACT exp [128,2048] ~2.0us; DVE schraudolph costs ~2.3us
# of DVE plus ~2.1us of gpsimd per [128,1024] half -- so ACT takes
# everything by default and DVE groups are an optional rebalance knob.
DVE_GROUPS = set()
DVE_EXP = {(h, jp): (h, jp) in DVE_GROUPS
           for h in range(HPC) for jp in range(2)}


def build_nc():
    nc = bacc.Bacc("TRN2", target_bir_lowering=False, debug=False,
                   num_devices=NCORES)
    xT = nc.declare_dram_parameter("xT", [HID, S], FP16, isOutput=False)
    wq = nc.declare_dram_parameter("wq", [HID, QC], FP16, isOutput=False)
    wk = nc.declare_dram_parameter("wk", [HID, QC], FP16, isOutput=False)
    wv = nc.declare_dram_parameter("wv", [HID, QC], FP16, isOutput=False)
    wo = nc.declare_dram_parameter("wo", [QC, HID], FP16, isOutput=False)
    bq = nc.declare_dram_parameter("bq", [QC], F32, isOutput=False)
    bk = nc.declare_dram_parameter("bk", [QC], F32, isOutput=False)
    ident = nc.declare_dram_parameter("ident", [128, 128], FP16, isOutput=False)
    out0 = nc.declare_dram_parameter("out0", [S, HID], FP16, isOutput=True)
    out1 = nc.declare_dram_parameter("out1", [S, HID], FP16, isOutput=True)

    with tile.TileContext(nc) as tc:
        with (
            tc.tile_pool(name="const", bufs=1) as constp,
            tc.tile_pool(name="big", bufs=1) as bigp,
            tc.tile_pool(name="vtst", bufs=3) as vtstp,
            tc.tile_pool(name="probs", bufs=6) as probsp,
            tc.tile_pool(name="schr", bufs=4) as schrp,
            tc.tile_pool(name="craw", bufs=3) as crawp,
            tc.tile_pool(name="div", bufs=2) as divp,
            tc.tile_pool(name="ostg", bufs=4) as ostg,
            tc.tile_pool(name="scps", bufs=1, space="PSUM") as scps,
            tc.tile_pool(name="ctps", bufs=1, space="PSUM") as ctps,
            tc.tile_pool(name="wkps", bufs=2, space="PSUM") as wkps,
        ):
            bq_sb = constp.tile([128, 2], F32)
            bk_sb = constp.tile([128, 2], F32)
            warm_sb = constp.tile([128, 1], F32)
            idh_sb = constp.tile([128, 128], FP16)
            wo_sb = constp.tile([128, 2 * HID], FP16)
            # preload the exp table set while the input DMAs run
            nc.vector.memset(warm_sb[:, :], 0.0)
            nc.scalar.activation(warm_sb[:, :], warm_sb[:, :], EXP)

            xT_sb = bigp.tile([128, HC * S], FP16)
            wq_sb = bigp.tile([128, HC * QC], FP16)
            wk_sb = bigp.tile([128, HC * QC], FP16)
            wv_sb = bigp.tile([128, HC * QC], FP16)
            qt2 = bigp.tile([128, HPC * S], FP16)
            kt2 = bigp.tile([128, HPC * S], FP16)
            # natural V in 128-wide per-(chunk, head) strips: cols 0:64 hold
            # the head's V dims, col 64 holds ones (so the M=65 ctx matmul
            # stationary also produces the softmax denominator in row 64).
            v_sb = bigp.tile([128, TC * HPC * 128], FP16)
            ctxf_sb = bigp.tile([128, 2 * S], FP16)
            for t in range(TC):
                for h in range(HPC):
                    off = (t * HPC + h) * 128 + HD
                    nc.vector.memset(v_sb[:, off:off + 1], 1.0)

            # ---- input DMAs: xT spread over 3 queues, ci0 weights first --
            qdma = [nc.sync, nc.scalar, nc.gpsimd, nc.sync]
            for hc in range(HC):
                r = slice(hc * 128, (hc + 1) * 128)
                eng = qdma[hc % 4]
                eng.dma_start(wq_sb[:, hc * QC:hc * QC + 128], wq[r, 0:128])
                eng.dma_start(wk_sb[:, hc * QC:hc * QC + 128], wk[r, 0:128])
                eng.dma_start(xT_sb[:, hc * S:(hc + 1) * S], xT[r, :])
            nc.sync.dma_start(idh_sb[:, :], ident[:, :])
            for ci in range(2):
                nc.sync.dma_start(bq_sb[:, ci:ci + 1],
                                  bq[ci * 128:(ci + 1) * 128])
                nc.sync.dma_start(bk_sb[:, ci:ci + 1],
                                  bk[ci * 128:(ci + 1) * 128])
            # later-needed weights: queue after the critical loads
            for hc in range(HC):
                r = slice(hc * 128, (hc + 1) * 128)
                qdma[hc % 4].dma_start(wv_sb[:, hc * QC:(hc + 1) * QC],
                                       wv[r, :])
            for hc in range(HC):
                r = slice(hc * 128, (hc + 1) * 128)
                qdma[(hc + 1) % 4].dma_start(
                    wq_sb[:, hc * QC + 128:hc * QC + 256], wq[r, 128:256])
                qdma[(hc + 2) % 4].dma_start(
                    wk_sb[:, hc * QC + 128:hc * QC + 256], wk[r, 128:256])
            for ci in range(2):
                qdma[1 + ci].dma_start(wo_sb[:, ci * HID:(ci + 1) * HID],
                                       wo[ci * 128:(ci + 1) * 128, :])

            # ---- building blocks -----------------------------------------
            def qk_unit(w_sb, b_sb, dst, ci, j):
                """Project one j-block of Q^T or K^T for ci's head pair into
                the duplicated per-head layout."""
                ps = wkps.tile([128, 512], F32, tag="wk")
                for hc in range(HC):
                    nc.tensor.matmul(
                        ps[:, :],
                        w_sb[:, hc * QC + ci * 128:hc * QC + ci * 128 + 128],
                        xT_sb[:, hc * S + j * 512:hc * S + j * 512 + 512],
                        start=(hc == 0), stop=(hc == HC - 1))
                hA, hB = 2 * ci, 2 * ci + 1
                jo = j * 512
                nc.scalar.activation(
                    dst[0:64, hA * S + jo:hA * S + jo + 512], ps[0:64, :],
                    IDENT_FN, bias=b_sb[0:64, ci:ci + 1])
                nc.vector.tensor_scalar_add(
                    dst[64:128, hB * S + jo:hB * S + jo + 512], ps[64:128, :],
                    b_sb[64:128, ci:ci + 1])
                nc.sync.dma_start(dst[64:128, hA * S + jo:hA * S + jo + 512],
                                  dst[0:64, hA * S + jo:hA * S + jo + 512])
                nc.scalar.dma_start(dst[0:64, hB * S + jo:hB * S + jo + 512],
                                    dst[64:128, hB * S + jo:hB * S + jo + 512])

            def vt_unit(ci, j):
                """V^T for one j-block of ci's head pair -> fp16 staging."""
                ps = wkps.tile([128, 512], F32, tag="wk")
                for hc in range(HC):
                    nc.tensor.matmul(
                        ps[:, :],
                        wv_sb[:, hc * QC + ci * 128:hc * QC + ci * 128 + 128],
                        xT_sb[:, hc * S + j * 512:hc * S + j * 512 + 512],
                        start=(hc == 0), stop=(hc == HC - 1))
                vt = vtstp.tile([128, 512], FP16, tag="vt",
                                name=f"vt_{ci}_{j}")
                nc.vector.tensor_copy(vt[:, :], ps[:, :])
                return vt

            def tr_unit(ci, t, vt):
                """Transpose one 128-token chunk of V^T into natural V: two
                contiguous 64-wide copies into the per-head strips."""
                tp = wkps.tile([128, 128], FP16, tag="wk")
                nc.tensor.transpose(tp[:, :],
                                    vt[:, (t % 4) * 128:(t % 4) * 128 + 128],
                                    idh_sb[:, :])
                for hl in range(2):
                    dst = (t * HPC + 2 * ci + hl) * 128
                    nc.vector.tensor_copy(v_sb[:, dst:dst + HD],
                                          tp[:, hl * HD:(hl + 1) * HD])

            def filler_units():
                for ci in range(2):
                    if ci == 1:
                        for j in range(TB):
                            yield lambda j=j: qk_unit(wq_sb, bq_sb, qt2, 1, j)
                            yield lambda j=j: qk_unit(wk_sb, bk_sb, kt2, 1, j)
                    for j in range(TB):
                        vt = []
                        yield lambda ci=ci, j=j, vt=vt: vt.append(
                            vt_unit(ci, j))
                        for tt in range(2):
                            yield lambda ci=ci, j=j, tt=tt, vt=vt: (
                                tr_unit(ci, j * 4 + 2 * tt, vt[0]),
                                tr_unit(ci, j * 4 + 2 * tt + 1, vt[0]))

            def v_stationary(ck, h):
                """[128, 65] stationary: head h's V dims for chunk ck plus
                the ones column -> ctx rows 0:64 + denominator in row 64."""
                base = (ck * HPC + h) * 128
                return v_sb[:, base:base + 65]

            probs_tiles = {}
            ctx_tiles = {}

            def emit_scores(h, jp, cp):
                hS = h * S
                c0, c1 = 2 * cp, 2 * cp + 1
                probs_c = probsp.tile([128, 2048], FP16, tag="probs",
                                      name=f"probs_h{h}_jp{jp}_cp{cp}")
                probs_tiles[(h, jp, cp)] = probs_c
                sp = scps.tile([128, 2048], F32, tag="sc")
                for jl in range(2):
                    j = 2 * jp + jl
                    nc.tensor.matmul(
                        sp[:, jl * 1024:jl * 1024 + 512],
                        kt2[0:64, hS + c0 * 128:hS + c0 * 128 + 128],
                        qt2[0:64, hS + j * 512:hS + j * 512 + 512],
                        start=True, stop=True)
                    nc.tensor.matmul(
                        sp[:, jl * 1024 + 512:jl * 1024 + 1024],
                        kt2[64:128, hS + c1 * 128:hS + c1 * 128 + 128],
                        qt2[64:128, hS + j * 512:hS + j * 512 + 512],
                        start=True, stop=True)
                if DVE_EXP[(h, jp)]:
                    for jl in range(2):
                        dst = probs_c[:, jl * 1024:(jl + 1) * 1024]
                        spl = sp[:, jl * 1024:(jl + 1) * 1024]
                        s1 = schrp.tile([128, 1024], I16, tag="schr")
                        nc.vector.tensor_scalar(
                            s1[:, :], spl, B16A, None, ADD)
                        nc.vector.tensor_scalar(
                            dst.bitcast(I16), spl, B16B, None, ADD)
                        nc.gpsimd.tensor_tensor(
                            out=dst, in0=dst, in1=s1[:, :].bitcast(FP16),
                            op=ADD)
                else:
                    nc.scalar.activation(probs_c[:, :], sp[:, :], EXP,
                                         scale=1.0 / A16)

            def emit_ctx(h, jp, cp):
                c0, c1 = 2 * cp, 2 * cp + 1
                if cp == 0:
                    ctx_tiles[(h, jp)] = ctps.tile(
                        [128, 1024], F32, tag="ctx", name=f"ctx_h{h}_jp{jp}")
                ctx_ps = ctx_tiles[(h, jp)]
                probs_c = probs_tiles.pop((h, jp, cp))
                for ck, coff in ((c0, 0), (c1, 512)):
                    lhsT = v_stationary(ck, h)
                    for jl in range(2):
                        nc.tensor.matmul(
                            ctx_ps[0:65, jl * 512:(jl + 1) * 512],
                            lhsT,
                            probs_c[:, jl * 1024 + coff:jl * 1024 + coff + 512],
                            start=(cp == 0 and ck == c0),
                            stop=(cp == TC // 2 - 1 and ck == c1))

            def emit_division(h, jp):
                ci = h // 2
                ctx_ps = ctx_tiles.pop((h, jp))
                craw = crawp.tile([128, 1024], F32, tag="craw")
                nc.vector.tensor_copy(craw[0:65, :], ctx_ps[0:65, :])
                denr = divp.tile([128, 8], F32, tag="denr")
                nc.sync.dma_start(denr[:, :], craw[64:65, :])
                recr = divp.tile([128, 8], F32, tag="recr")
                nc.vector.reciprocal(recr[:], denr[:])
                rrow = divp.tile([1, 1024], F32, tag="rrow")
                nc.sync.dma_start(rrow[:, :], recr[:, :])
                Dt = divp.tile([128, 1024], F32, tag="Dt")
                nc.gpsimd.partition_broadcast(Dt[:, :], rrow[0:1, :])
                dst_c = ci * S + jp * 1024
                if h % 2 == 0:
                    nc.gpsimd.tensor_tensor(
                        out=ctxf_sb[0:64, dst_c:dst_c + 1024],
                        in0=craw[0:64, :], in1=Dt[0:64, :], op=MULT)
                else:
                    ctxd = crawp.tile([64, 1024], FP16, tag="ctxd")
                    nc.gpsimd.tensor_tensor(
                        out=ctxd[0:64, :],
                        in0=craw[0:64, :], in1=Dt[0:64, :], op=MULT)
                    nc.sync.dma_start(
                        ctxf_sb[64:128, dst_c:dst_c + 1024], ctxd[0:64, :])

            def emit_outproj(ci, t, out_t, copy_eng):
                ot = ostg.tile([128, 1024], FP16, tag="ot")
                for oc in range(2):
                    op = wkps.tile([128, 512], F32, tag="wk")
                    nc.tensor.matmul(
                        op[:, :],
                        ctxf_sb[:, ci * S + t * 128:ci * S + t * 128 + 128],
                        wo_sb[:, ci * HID + oc * 512:ci * HID + oc * 512 + 512],
                        start=True, stop=True)
                    if copy_eng == "act":
                        nc.scalar.activation(
                            ot[:, oc * 512:(oc + 1) * 512], op[:, :], COPY_FN)
                    else:
                        nc.vector.tensor_copy(
                            ot[:, oc * 512:(oc + 1) * 512], op[:, :])
                nc.sync.dma_start(out_t[t * 128:(t + 1) * 128, :], ot[:, :])

            # ---- emission ------------------------------------------------
            # ci0 Q/K first so scores start as soon as xT lands
            for j in range(TB):
                qk_unit(wq_sb, bq_sb, qt2, 0, j)
                qk_unit(wk_sb, bk_sb, kt2, 0, j)

            # stage list: ci0 heads first; the last head pair interleaved by
            # jp so the ci1/jp0 quarter of ctxf completes early.
            NCP = TC // 2
            stages = ([(1, jp, cp) for jp in range(2) for cp in range(NCP)] +
                      [(0, jp, cp) for jp in range(2) for cp in range(NCP)] +
                      [(3, 0, cp) for cp in range(NCP)] +
                      [(2, 0, cp) for cp in range(NCP)] +
                      [(3, 1, cp) for cp in range(NCP)] +
                      [(2, 1, cp) for cp in range(NCP)])
            fillers = filler_units()
            LEAD = 2
            nslot = 0

            def pump_outproj(i):
                nonlocal nslot
                # ci0 ready after stage 31 (+LEAD+division latency), ci1/jp0
                # after stage 47, ci1/jp1 after stage 63.
                if i >= 36 and nslot < 16:
                    t = nslot
                    emit_outproj(0, t, out0, "dve")
                    nslot += 1
                elif i >= 52 and nslot < 24:
                    t = nslot - 16
                    emit_outproj(1, t, out1, "dve")
                    nslot += 1

            filler_budget = [0, 2, 2, 2, 2, 2] + [1] * 44
            for i in range(len(stages) + LEAD):
                # PE work that covers the in-flight exp comes first, so the
                # single-buffered scores tile never stalls the PE.
                if i >= LEAD:
                    h, jp, cp = stages[i - LEAD]
                    emit_ctx(h, jp, cp)
                    if cp == NCP - 1:
                        emit_division(h, jp)
                    pump_outproj(i)
                    pump_outproj(i)
                if i < len(stages):
                    nb = filler_budget[i] if i < len(filler_budget) else 0
                    for _ in range(nb):
                        u = next(fillers, None)
                        if u is not None:
                            u()
                    emit_scores(*stages[i])
            for u in fillers:
                u()
            while nslot < 32:
                emit_outproj(1, nslot - 16, out1, "dve")
                nslot += 1

    nc.compile()
    return nc


_NC = None


def _get_nc():
    global _NC
    if _NC is None:
        _NC = build_nc()
    return _NC


def make_in_maps(x, Wq, bq, Wk, bk, Wv, bv, Wo, bo):
    qscale = 0.125 * A16
    in_maps = []
    for core in range(NCORES):
        b, g = core // 4, core % 4
        sl = slice(g * QC, (g + 1) * QC)
        in_maps.append({
            "xT": np.ascontiguousarray(x[b].T).astype(np.float16),
            "wq": (np.ascontiguousarray(Wq[:, sl]) * qscale).astype(np.float16),
            "wk": np.ascontiguousarray(Wk[:, sl]).astype(np.float16),
            "wv": np.ascontiguousarray(Wv[:, sl]).astype(np.float16),
            "wo": np.ascontiguousarray(Wo[sl, :]).astype(np.float16),
            "bq": (np.asarray(bq[sl]) * qscale).astype(np.float32),
            "bk": np.asarray(bk[sl]).astype(np.float32),
            "ident": np.eye(128, dtype=np.float16),
        })
    return in_maps


def combine_outputs(core_outs, Wv_bias_term):
    full = np.empty((B, S, HID), np.float32)
    for b in range(B):
        acc = core_outs[4 * b][0].astype(np.float32)
        acc += core_outs[4 * b][1]
        for g in range(1, 4):
            acc += core_outs[4 * b + g][0]
            acc += core_outs[4 * b + g][1]
        full[b] = acc + Wv_bias_term
    return full


def kernel(**inputs):
    x = np.asarray(inputs["x"], np.float32)
    Wq = np.asarray(inputs["Wq"], np.float32)
    bq = np.asarray(inputs["bq"], np.float32)
    Wk = np.asarray(inputs["Wk"], np.float32)
    bk = np.asarray(inputs["bk"], np.float32)
    Wv = np.asarray(inputs["Wv"], np.float32)
    bv = np.asarray(inputs["bv"], np.float32)
    Wo = np.asarray(inputs["Wo"], np.float32)
    bo = np.asarray(inputs["bo"], np.float32)

    nc = _get_nc()
    in_maps = make_in_maps(x, Wq, bq, Wk, bk, Wv, bv, Wo, bo)
    res = run_bass_kernel_spmd(nc, in_maps, core_ids=list(range(NCORES)))
    core_outs = [(res.results[c]["out0"], res.results[c]["out1"])
                 for c in range(NCORES)]
    bias_term = (bv @ Wo + bo).astype(np.float32)
    return combine_outputs(core_outs, bias_term)
